# revision 1
# baseline (speedup 1.0000x reference)
"""MoE ConditionalFeedForward (gated SiLU FFN with top-2 routing) on 8 TRN2
NeuronCores.

Problem: x:(2048,2048) f32, expert_indices:(2048,2) int32 in [0,8),
w1/w3:(8,4096,2048), w2:(8,2048,4096). out[t,a,:] = FFN_{e=idx[t,a]}(x[t])
with FFN_e(v) = (silu(w1[e]@v) * (w3[e]@v)) @ w2[e].T.

Sharding: expert-parallel. Core e owns expert e's weights. The host gathers
the tokens routed to each expert (the "all-to-all dispatch" of the sharding
hint, done at shard time), pads to a common capacity C, and each core runs a
dense fused FFN over its token batch:

    h   = silu(w1 x_e) * (w3 x_e)      [Inter x C, feature-major]
    out = w2 h                          [D x C]

All matmuls run as float32r (TRN2 reduced-precision fp32 mode, 1 col/cycle;
measured same accuracy as the fp32 4-cycle mode on this silicon). Data is
pre-arranged on the host so every DMA is a plain 2D copy (3D-view DMA
destinations fail on hardware via this execution path).

Layouts (per core, K-major feature-on-partitions):
    xt   [128, 16*C]   xt[p, k*C+c]    = x_e[c, k*128+p]
    w1t  [32, 128, 2048] w1t[ib,p,k*128+j] = w1[e, ib*128+j, k*128+p]
    w3t  same as w1t
    w2t  [16, 128, 4096] w2t[db,p,k2*128+j] = w2[e, db*128+j, k2*128+p]
    ot   [16, 128, C]  ot[db,p,c]      = out_e[db*128+p, c]
"""

import numpy as np

T, A, E, D, INTER = 2048, 2, 8, 2048, 4096
P = 128
NK1 = D // P       # 16  k-tiles for GEMM1/3
NM1 = INTER // P   # 32  m-tiles for GEMM1/3 (= k-tiles for GEMM2)
NM2 = D // P       # 16  m-tiles for GEMM2


def _chunks(C):
    """Split C into contiguous chunks of <=512 (PSUM bank limit), each a
    multiple of 8 and >=256 when possible (fp32r full-rate needs >=256)."""
    n = (C + 511) // 512
    base = C // n // 8 * 8
    out = []
    off = 0
    for i in range(n):
        w = base if i < n - 1 else C - off
        out.append((off, w))
        off += w
    return out


def build_ffn(C, with_reps=False, max_reps=4096):
    import concourse.bacc as bacc
    import concourse.mybir as mybir
    from concourse import tile

    f32 = mybir.dt.float32
    f32r = mybir.dt.float32r

    nc = bacc.Bacc(None, target_bir_lowering=False, debug=False)

    xt = nc.dram_tensor("xt", [P, NK1 * C], f32r, kind="ExternalInput")
    w1t = nc.dram_tensor("w1t", [NM1, P, NK1 * P], f32r, kind="ExternalInput")
    w3t = nc.dram_tensor("w3t", [NM1, P, NK1 * P], f32r, kind="ExternalInput")
    w2t = nc.dram_tensor("w2t", [NM2, P, NM1 * P], f32r, kind="ExternalInput")
    ot = nc.dram_tensor("ot", [NM2, P, C], f32, kind="ExternalOutput")
    if with_reps:
        reps = nc.dram_tensor("reps", [1, 1], mybir.dt.int32, kind="ExternalInput")

    chunks = _chunks(C)
    silu = mybir.ActivationFunctionType.Silu

    with tile.TileContext(nc) as tc:
        with (
            tc.tile_pool(name="xpool", bufs=1) as xpool,
            tc.tile_pool(name="hpool", bufs=1) as hpool,
            tc.tile_pool(name="w13", bufs=2) as w13pool,
            tc.tile_pool(name="w2p", bufs=2) as w2pool,
            tc.tile_pool(name="tmp", bufs=3) as tmppool,
            tc.tile_pool(name="outp", bufs=3) as outpool,
            tc.tile_pool(name="cst", bufs=1) as cstpool,
            tc.tile_pool(name="ps1", bufs=2, space="PSUM") as ps1pool,
            tc.tile_pool(name="ps3", bufs=2, space="PSUM") as ps3pool,
            tc.tile_pool(name="ps2", bufs=2, space="PSUM") as ps2pool,
        ):
            sb_x = xpool.tile([P, NK1 * C], f32r)
            nc.sync.dma_start(out=sb_x[:], in_=xt[:])
            sb_h = hpool.tile([P, NM1 * C], f32r)

            def body():
                # Phase 1: h = silu(w1 x) * (w3 x), laid out [Inter, C]
                for ib in range(NM1):
                    tw1 = w13pool.tile([P, NK1 * P], f32r, tag="w1")
                    nc.sync.dma_start(out=tw1[:], in_=w1t[ib])
                    tw3 = w13pool.tile([P, NK1 * P], f32r, tag="w3")
                    nc.sync.dma_start(out=tw3[:], in_=w3t[ib])
                    for (c0, cw) in chunks:
                        ps1 = ps1pool.tile([P, cw], f32, tag="ps1")
                        for k in range(NK1):
                            nc.tensor.matmul(
                                ps1[:], lhsT=tw1[:, k * P:(k + 1) * P],
                                rhs=sb_x[:, k * C + c0: k * C + c0 + cw],
                                start=(k == 0), stop=(k == NK1 - 1))
                        ps3 = ps3pool.tile([P, cw], f32, tag="ps3")
                        for k in range(NK1):
                            nc.tensor.matmul(
                                ps3[:], lhsT=tw3[:, k * P:(k + 1) * P],
                                rhs=sb_x[:, k * C + c0: k * C + c0 + cw],
                                start=(k == 0), stop=(k == NK1 - 1))
                        tmp = tmppool.tile([P, cw], f32, tag="tmp")
                        nc.scalar.activation(tmp[:], ps1[:], silu)
                        nc.vector.tensor_mul(
                            sb_h[:, ib * C + c0: ib * C + c0 + cw],
                            tmp[:], ps3[:])

                # Phase 2: out = w2 h, laid out [D, C]
                for db in range(NM2):
                    tw2 = w2pool.tile([P, NM1 * P], f32r, tag="w2")
                    nc.sync.dma_start(out=tw2[:], in_=w2t[db])
                    for (c0, cw) in chunks:
                        ps2 = ps2pool.tile([P, cw], f32, tag="ps2")
                        for k in range(NM1):
                            nc.tensor.matmul(
                                ps2[:], lhsT=tw2[:, k * P:(k + 1) * P],
                                rhs=sb_h[:, k * C + c0: k * C + c0 + cw],
                                start=(k == 0), stop=(k == NM1 - 1))
                        to = outpool.tile([P, cw], f32, tag="to")
                        nc.vector.tensor_copy(to[:], ps2[:])
                        nc.sync.dma_start(out=ot[db, :, c0:c0 + cw], in_=to[:])

            if with_reps:
                rt = cstpool.tile([1, 1], mybir.dt.int32, tag="rt")
                nc.sync.dma_start(out=rt[:], in_=reps[:])
                rregs = nc.alloc_registers("reps_regs")
                nc.regs_load(rregs, rt[0:1, 0:1])
                nval = nc.snap(rregs, donate=True, min_val=1, max_val=max_reps)
                with tc.For_i(0, nval, 1):
                    body()
                    # gpsimd must participate in the loop for For_i
                    gp = cstpool.tile([1, 4], mybir.dt.float32, tag="gp")
                    nc.gpsimd.memset(gp[:], 0.0)
            else:
                body()
    nc.compile()
    return nc


def shard_inputs(x, expert_indices, w1, w2, w3):
    """Host-side routing + layout. Returns (in_maps, meta) where meta has
    what's needed to unshard."""
    flat = np.asarray(expert_indices).ravel()          # flat index f = t*A + a
    order = np.argsort(flat, kind="stable")
    counts = np.bincount(flat, minlength=E)
    C = max(512, int(-(-counts.max() // 16) * 16))     # capacity, mult of 16
    bounds = np.zeros(E + 1, np.int64)
    np.cumsum(counts, out=bounds[1:])

    xf = np.asarray(x, np.float32)
    in_maps = []
    per_core_ids = []
    for e in range(E):
        ids = order[bounds[e]:bounds[e + 1]]
        per_core_ids.append(ids)
        tok = ids // A
        xg = np.zeros((C, D), np.float32)
        xg[:len(tok)] = xf[tok]
        # xt[p, k*C+c] = xg[c, k*128+p]
        xt = np.ascontiguousarray(
            xg.T.reshape(NK1, P, C).transpose(1, 0, 2).reshape(P, NK1 * C))
        w1t = np.ascontiguousarray(
            np.asarray(w1[e], np.float32).reshape(NM1, P, NK1, P)
            .transpose(0, 3, 2, 1).reshape(NM1, P, NK1 * P))
        w3t = np.ascontiguousarray(
            np.asarray(w3[e], np.float32).reshape(NM1, P, NK1, P)
            .transpose(0, 3, 2, 1).reshape(NM1, P, NK1 * P))
        w2t = np.ascontiguousarray(
            np.asarray(w2[e], np.float32).reshape(NM2, P, NM1, P)
            .transpose(0, 3, 2, 1).reshape(NM2, P, NM1 * P))
        in_maps.append({"xt": xt, "w1t": w1t, "w3t": w3t, "w2t": w2t})
    return in_maps, (per_core_ids, C)


def unshard_output(results, meta):
    per_core_ids, C = meta
    out = np.zeros((T, A, D), np.float32)
    for e in range(E):
        ids = per_core_ids[e]
        if len(ids) == 0:
            continue
        om = results[e]["ot"].reshape(D, C)            # [d, c]
        out.reshape(T * A, D)[ids] = om[:, :len(ids)].T
    return out


def kernel(x, expert_indices, w1, w2, w3):
    from concourse.bass_utils import run_bass_kernel_spmd

    in_maps, meta = shard_inputs(x, expert_indices, w1, w2, w3)
    nc = build_ffn(meta[1], with_reps=False)
    res = run_bass_kernel_spmd(nc, in_maps, core_ids=list(range(E)), trace=False)
    return unshard_output(res.results, meta)
